# revision 1
# baseline (speedup 1.0000x reference)
"""Correntropy loss on 8 Trainium2 NeuronCores — fp8 gram + fp16 sampled
bias correction.

Reference math (all f32):
    t = (target - 0.5) * 2 ; o = (output - 0.5) * 2
    cost = mean(1 - exp(-sigma * (o - t)^2)),  sigma = 1/1000
Since o - t == 2*(output - target), this equals
    mean(1 - exp(-c * w)),  w = (output - target)^2,  c = 4*sigma = 0.004

The kernel is pure HBM-bandwidth bound, so the host stages most of the
device buffers in float8-e4m3 (4x less traffic than f32).  fp8
round-to-nearest gives S1 a deterministic quantization bias (~4e-3
relative — inside but uncomfortably close to the 2e-2 tolerance), so
two of the 16 row-tiles are shipped BOTH as fp16 and as fp8 and the
bias is estimated from the sample and removed:
    S1 = S1_fp8(12 tiles) + 7*S1_fp16(2 tiles) - 6*S1_fp8(same 2)
         + S1_fp16(2 plain tiles)
(simulated end-to-end error vs the f32 reference: 1.3e-5).

Device computes power sums of w; host evaluates the 1-exp Taylor
series in f64:  sum(1-exp(-c*w)) = c*S1 - c^2/2*S2 + O(c^3*S3).
S2 (a 3.2e-3 relative correction) comes from the 2 fp16 sample tiles
scaled by 8; the dropped S3 term is +9.1e-6 relative.

Engine layout.  fp8 tiles are host-packed as 63 chunks of
[o(64)|t(64)] (zero-padded) and TensorE runs a self-gram
matmul(C += blk.T @ blk) per 128-column block; the diagonal families
of C give sum o^2, sum t^2, sum o*t, hence sum (o-t)^2, with NO
DVE/ACT work.  The 14 fp8 gram tiles (2 sample duplicates into PSUM
bank C2, then 12 population tiles into C1) are contiguous at the
stream front, each split into two half-pieces for finer DMA->PE
overlap: a saturated PE latches the HAM clock gate to 2.4 GHz (56 ns
per 128x128 block, weight loads hidden), making the PE phase ~55 us;
a warm-up burst of matmuls on a zeroed tile latches the gate before
the first data arrives.  The four fp16 tiles ([o(z)|t(z)] halves; DVE sub ->
ACT Square accum, the sample tiles adding a second Square for S2) ride
the spare DMA bandwidth during the PE-paced phase: their DMAs issue
from the otherwise-idle GPSIMD queue so a PE-gated gram DMA can never
block them, and their compute hides entirely.

Outputs: two [128,128] gram matrices (PSUM -> SBUF copy on the scalar
engine, written out mid-stream) and the ACT accumulator columns; the
host reduces everything in f64 and applies the series (the scalar
"all-reduce" of the sharding hint, done exactly on the host).
"""

import numpy as np

import concourse.bacc as bacc
import concourse.mybir as mybir
import concourse.tile as tile
from concourse.bass_utils import run_bass_kernel_spmd

N_CORES = 8
ROWS = 65536
COLS = 1000
ROWS_PER_CORE = ROWS // N_CORES  # 8192
P = 128  # SBUF partitions

Q = 4  # rows folded into the free dim per partition
FREE = Q * COLS  # 4000 elements of one operand per partition per tile
N_TILES = ROWS_PER_CORE // (P * Q)  # 16

CH = 64  # gram chunk width per operand
N_CHUNK = -(-FREE // CH)  # 63 chunks (last one zero-padded)
GFREE = N_CHUNK * CH  # 4032 padded columns per operand
BLK = 2 * CH  # 128-wide [o64|t64] gram block

# Data-tile roles (by row-tile index 0..15):
SAMPLE_TILES = (0, 1)   # shipped fp8 (into C2) AND fp16 (ACT path + S2)
FP16_TILES = (14, 15)   # shipped fp16 only (ACT path)
GRAM_TILES = tuple(t for t in range(N_TILES)
                   if t not in SAMPLE_TILES and t not in FP16_TILES)  # 12
S2_SCALE = float(N_TILES) / len(SAMPLE_TILES)  # 8.0
CORR_SCALE = float(len(GRAM_TILES)) / len(SAMPLE_TILES)  # 6.0

# Stream pieces, in DMA order.  kind: "g8" fp8 gram piece (nchunk
# chunks), "a16" fp16 ACT piece.  The two sample duplicates lead (C2
# group), then the 12 population gram tiles (C1 group) with the last
# tile split in half to shorten the post-stream PE chain.  The fp16
# pieces are interleaved mid-stream on the GPSIMD DMA queue.
# Every fp8 tile is split into two half-pieces (32+31 chunks) so the
# PE's per-piece DMA-completion wait covers half the bytes and overlap
# with the stream is finer-grained.
# The small C2 (sample-duplicate) pieces go LAST: the big C1 group then
# closes and writes back mid-stream (hidden), and the post-stream chain
# is just the final 15-chunk piece's matmuls + C2 copy + writeback.
PIECES = []
for _t in GRAM_TILES:
    PIECES.append(("g8", _t, 0, 32))
    PIECES.append(("g8", _t, 32, N_CHUNK - 32))
PIECES.append(("d8", SAMPLE_TILES[0], 0, 32))
PIECES.append(("d8", SAMPLE_TILES[0], 32, N_CHUNK - 32))
PIECES.append(("d8", SAMPLE_TILES[1], 0, 48))
PIECES.append(("d8", SAMPLE_TILES[1], 48, N_CHUNK - 48))
# fp16 pieces (gpsimd queue; stream position chosen by slot availability)
A16_PIECES = [("a16", SAMPLE_TILES[0], None, FREE),
              ("a16", SAMPLE_TILES[1], None, FREE),
              ("a16", FP16_TILES[0], None, FREE),
              ("a16", FP16_TILES[1], None, FREE)]
N_A16 = len(A16_PIECES)
ACC_COLS = 2 * N_A16  # S1 cols | S2 cols (S2 used for sample pieces)
N_MM_C2 = 2 * N_CHUNK  # 126
N_MM_C1 = 12 * N_CHUNK  # 756

F32 = mybir.dt.float32
F16 = mybir.dt.float16
F8 = mybir.dt.float8e4


def _build():
    nc = bacc.Bacc()
    comb8_elems = sum(P * 2 * CH * n for k, t, o, n in PIECES)
    comb8_p = nc.declare_dram_parameter("comb8", [comb8_elems], F8, isOutput=False)
    comb16_p = nc.declare_dram_parameter(
        "comb16", [N_A16 * P * 2 * FREE], F16, isOutput=False
    )
    acc_p = nc.declare_dram_parameter("partial", [P, ACC_COLS], F32, isOutput=True)
    gram1_p = nc.declare_dram_parameter("gram1", [BLK, BLK], F32, isOutput=True)
    gram2_p = nc.declare_dram_parameter("gram2", [BLK, BLK], F32, isOutput=True)

    with tile.TileContext(nc) as tc:
        with (
            tc.tile_pool(name="io", bufs=6) as io_pool,
            tc.tile_pool(name="work", bufs=1) as work_pool,
            tc.tile_pool(name="accp", bufs=1) as acc_pool,
            tc.psum_pool(name="gr", bufs=2) as psum_pool,
        ):
            acc = acc_pool.tile([P, ACC_COLS], F32)
            gram1 = psum_pool.tile([BLK, BLK], F32, tag="g1")
            gram2 = psum_pool.tile([BLK, BLK], F32, tag="g2")
            gram1_sb = acc_pool.tile([BLK, BLK], F32)
            gram2_sb = acc_pool.tile([BLK, BLK], F32)

            # PE warm-up: back-to-back matmuls on a zeroed tile keep
            # the PE busy past the HAM's 3.4 us activity window, latching
            # the clock gate to 2.4 GHz before the first data arrives
            # (cold 128-row blocks take 107 ns vs 56 ns warm).
            warm = acc_pool.tile([P, BLK], F8)
            wpsum = psum_pool.tile([BLK, BLK], F32, tag="gw")
            nc.vector.memset(warm[:], 0)
            for wi in range(64):
                nc.tensor.matmul(
                    wpsum[:], warm[:], warm[:],
                    start=(wi == 0), stop=(wi == 63),
                )

            # fp16 side: DMAs on the GPSIMD queue, compute on DVE/ACT.
            a16_tiles = []
            ofs16 = 0
            for j, (k, t, off, z) in enumerate(A16_PIECES):
                ab = io_pool.tile([P, 2 * z], F16, tag="aba", bufs=1)
                nc.gpsimd.dma_start(
                    out=ab[:],
                    in_=comb16_p[ofs16 : ofs16 + P * 2 * z].rearrange(
                        "(p m) -> p m", p=P
                    ),
                )
                ofs16 += P * 2 * z
                a16_tiles.append(ab)

            # fp8 gram stream on the sync queue.
            mm_c1 = mm_c2 = 0
            ofs8 = 0
            for k, t, off, nchunk in PIECES:
                z = 2 * CH * nchunk
                ab = io_pool.tile([P, z], F8, tag="abg", bufs=20)
                nc.sync.dma_start(
                    out=ab[:],
                    in_=comb8_p[ofs8 : ofs8 + P * z].rearrange("(p m) -> p m", p=P),
                )
                ofs8 += P * z
                if k == "d8":
                    for b in range(nchunk):
                        blk = ab[:, b * BLK : (b + 1) * BLK]
                        nc.tensor.matmul(
                            gram2[:], blk, blk,
                            start=(mm_c2 == 0), stop=(mm_c2 == N_MM_C2 - 1),
                        )
                        mm_c2 += 1
                    if mm_c2 == N_MM_C2:
                        nc.scalar.copy(gram2_sb[:], gram2[:])
                        nc.scalar.dma_start(out=gram2_p[:], in_=gram2_sb[:])
                else:
                    for b in range(nchunk):
                        blk = ab[:, b * BLK : (b + 1) * BLK]
                        nc.tensor.matmul(
                            gram1[:], blk, blk,
                            start=(mm_c1 == 0), stop=(mm_c1 == N_MM_C1 - 1),
                        )
                        mm_c1 += 1
                    if mm_c1 == N_MM_C1:
                        nc.scalar.copy(gram1_sb[:], gram1[:])
                        nc.scalar.dma_start(out=gram1_p[:], in_=gram1_sb[:])

            # fp16 compute (hides under the PE-paced fp8 phase).
            for j, (k, t, off, z) in enumerate(A16_PIECES):
                ab = a16_tiles[j]
                d = work_pool.tile([P, z], F16, tag="d", bufs=2)
                nc.vector.tensor_sub(d[:], ab[:, 0:z], ab[:, z : 2 * z])
                w = work_pool.tile([P, z], F16, tag="w", bufs=2)
                nc.scalar.activation(
                    w[:], d[:],
                    mybir.ActivationFunctionType.Square,
                    accum_out=acc[:, j : j + 1],
                )
                if t in SAMPLE_TILES:
                    w2 = work_pool.tile([P, z], F16, tag="w2", bufs=2)
                    nc.scalar.activation(
                        w2[:], w[:],
                        mybir.ActivationFunctionType.Square,
                        accum_out=acc[:, N_A16 + j : N_A16 + j + 1],
                    )
            nc.sync.dma_start(out=acc_p[:], in_=acc[:])
    nc.finalize()
    return nc


_NC = None


def _get_nc():
    global _NC
    if _NC is None:
        _NC = _build()
    return _NC


def _pack_gram_cols(o_t, t_t, c0, nchunk):
    """fp8 chunks [c0, c0+nchunk) of a row-tile -> [P, nchunk*BLK]."""
    pad = GFREE - FREE
    o_p = np.pad(o_t, ((0, 0), (0, pad))).reshape(P, N_CHUNK, CH)
    t_p = np.pad(t_t, ((0, 0), (0, pad))).reshape(P, N_CHUNK, CH)
    sel = slice(c0, c0 + nchunk)
    return np.stack([o_p[:, sel], t_p[:, sel]], axis=2).reshape(P, nchunk * BLK)


def _shard_inputs(output, target):
    import ml_dtypes  # noqa: F401  (float8 numpy dtype support)

    output = np.asarray(output)
    target = np.asarray(target)
    f8np = mybir.dt.np(F8)
    in_maps = []
    for ci in range(N_CORES):
        sl = slice(ci * ROWS_PER_CORE, (ci + 1) * ROWS_PER_CORE)
        o16 = output[sl].astype(np.float16).reshape(N_TILES, P, FREE)
        t16 = target[sl].astype(np.float16).reshape(N_TILES, P, FREE)
        o8 = output[sl].astype(f8np).reshape(N_TILES, P, FREE)
        t8 = target[sl].astype(f8np).reshape(N_TILES, P, FREE)
        blocks8 = []
        for k, t, c0, nchunk in PIECES:
            blocks8.append(_pack_gram_cols(o8[t], t8[t], c0, nchunk).reshape(-1))
        blocks16 = []
        for k, t, off, z in A16_PIECES:
            blk = np.concatenate([o16[t], t16[t]], axis=1)
            blocks16.append(blk.reshape(-1))
        in_maps.append(
            {
                "comb8": np.concatenate(blocks8),
                "comb16": np.concatenate(blocks16),
            }
        )
    return in_maps


def run_device(output, target, trace=False):
    in_maps = _shard_inputs(output, target)
    res = run_bass_kernel_spmd(_get_nc(), in_maps, list(range(N_CORES)), trace=trace)
    partials = [
        (
            res.results[i]["partial"],
            res.results[i]["gram1"],
            res.results[i]["gram2"],
        )
        for i in range(N_CORES)
    ]
    return partials, res


def _gram_s1(g64):
    dg = np.diag(g64)
    return dg[:CH].sum() + dg[CH:].sum() - 2.0 * np.diag(g64[:CH, CH:]).sum()


def _reduce(partials):
    s1 = s2 = 0.0
    for p, g1, g2 in partials:
        p64 = p.astype(np.float64)
        c1 = _gram_s1(g1.astype(np.float64))
        c2 = _gram_s1(g2.astype(np.float64))
        s1_fp16_sample = p64[:, 0].sum() + p64[:, 1].sum()
        s1_fp16_plain = p64[:, 2].sum() + p64[:, 3].sum()
        s1 += c1 + (CORR_SCALE + 1.0) * s1_fp16_sample - CORR_SCALE * c2
        s1 += s1_fp16_plain
        s2 += p64[:, N_A16 + 0].sum() + p64[:, N_A16 + 1].sum()
    s2 *= S2_SCALE
    c = 4.0 * float(np.float32(1.0 / COLS))  # match reference's f32 sigma
    total = c * s1 - (c * c / 2.0) * s2
    n = float(ROWS) * float(COLS)
    return np.array(total / n, dtype=np.float32)


def kernel(output, target):
    partials, _ = run_device(output, target)
    return _reduce(partials)



# revision 2
# speedup vs baseline: 1.3873x; 1.3873x over previous
"""Correntropy loss on 8 Trainium2 NeuronCores — centered-fp8, PE+ACT split.

Reference math (all f32):
    t = (target - 0.5) * 2 ; o = (output - 0.5) * 2
    cost = mean(1 - exp(-sigma * (o - t)^2)),  sigma = 1/1000
Since o - t == 2*(output - target) this equals
    mean(1 - exp(-c * w)),  w = (output - target)^2,  c = 4*sigma = 0.004

The kernel is HBM-bandwidth bound (per-core ceiling ~358 GB/s, ~375
measured), so the host ships each tensor at 1 byte/element: fp8-e4m3 of
(x - 0.5).  Centering before quantization is free (the difference o - t
is shift-invariant) and cuts the fp8 round-to-nearest bias on
sum (o-t)^2 from ~4.5e-3 to ~1.1e-3 relative (numpy-simulated
end-to-end vs the f32 reference; tolerance is 2e-2), so no bias
correction, fp16 sample tiles, or duplicated traffic are needed.
Per-core traffic: 16.46 MB (was 22.6 MB in the fp8+fp16 scheme).

Compute is split so no engine gates the stream (PE alone consumes fp8
at only ~256 GB/s, less than DMA delivers):
  * 9 of 16 row-tiles go to TensorE as a self-gram: host packs 63
    chunks of [o(64)|t(64)] per tile and C += blk.T @ blk accumulates;
    diagonal families of C give sum o^2, sum t^2, sum o*t, hence
    sum (o-t)^2, with no DVE/ACT work.  ~36 us of PE time.
  * 7 tiles go to the otherwise-idle DVE+ACT pair: host packs
    [o(4000)|t(4000)] per partition, DVE subtracts (fp8 -> fp16 d),
    ACT squares with a free accumulate into an SBUF column.  ~15 us
    DVE + ~36 us ACT, both hidden under the DMA stream.
Device computes power sums of w; the host evaluates the 1-exp Taylor
series in f64:  sum(1-exp(-c*w)) = c*S1 - c^2/2*S2 + O(c^3*S3).
S2 (an ~8e-4 relative correction) comes from a second ACT Square pass
on 2 of the 7 ACT tiles, scaled by 8; dropped S3 is ~1e-5 relative.

All input pieces ride ONE DMA queue (sync) in an explicit interleave
(ACT tile, then the matching PE tile as two half-pieces) so bandwidth
goes to each consumer just-in-time; the stream ends with pure-PE
pieces whose drain is ~2 us.  A warm-up burst of matmuls on a zeroed
tile latches the HAM clock gate to 2.4 GHz before the first data
arrives.  Outputs: one [128,128] gram (PSUM -> SBUF copy on DVE) and
9 ACT accumulator columns; the host reduces in f64 and applies the
series (the scalar "all-reduce" of the sharding hint, done exactly on
the host).
"""

import numpy as np

import concourse.bacc as bacc
import concourse.mybir as mybir
import concourse.tile as tile
from concourse.bass_utils import run_bass_kernel_spmd

N_CORES = 8
ROWS = 65536
COLS = 1000
ROWS_PER_CORE = ROWS // N_CORES  # 8192
P = 128  # SBUF partitions

Q = 4  # rows folded into the free dim per partition
FREE = Q * COLS  # 4000 elements of one operand per partition per tile
N_TILES = ROWS_PER_CORE // (P * Q)  # 16

CH = 64  # gram chunk width per operand
N_CHUNK = -(-FREE // CH)  # 63 chunks (last one zero-padded)
GFREE = N_CHUNK * CH  # 4032 padded columns per operand
BLK = 2 * CH  # 128-wide [o64|t64] gram block

N_PE_TILES = 9   # row-tiles 0..8: TensorE self-gram path
N_ACT_TILES = 7  # row-tiles 9..15: DVE sub -> ACT Square path
N_S2 = 2         # first N_S2 ACT tiles also get a second Square (S2)
S2_SCALE = float(N_TILES) / N_S2  # 8.0
ACC_COLS = N_ACT_TILES + N_S2  # 7 S1 cols | 2 S2 cols

# DMA stream order: byte-proportional interleave (ACT tile, then the
# matching PE tile split into two half-pieces for finer DMA->PE
# overlap), stream closed by the 3 remaining PE tiles so the ACT
# engine's last data arrives ~8 us before stream end.
STREAM = []
for _i in range(N_ACT_TILES):
    STREAM.append(("act", N_PE_TILES + _i, 0, 0))
    STREAM.append(("pe", _i, 0, 32))
    STREAM.append(("pe", _i, 32, N_CHUNK - 32))
for _i in range(N_ACT_TILES, N_PE_TILES):
    STREAM.append(("pe", _i, 0, 32))
    STREAM.append(("pe", _i, 32, N_CHUNK - 32))

N_MM = N_PE_TILES * N_CHUNK  # 567 gram matmuls

F32 = mybir.dt.float32
F16 = mybir.dt.float16
F8 = mybir.dt.float8e4


def _build():
    nc = bacc.Bacc()
    pe_elems = N_PE_TILES * P * 2 * GFREE
    act_elems = N_ACT_TILES * P * 2 * FREE
    pe_p = nc.declare_dram_parameter("comb_pe", [pe_elems], F8, isOutput=False)
    act_p = nc.declare_dram_parameter("comb_act", [act_elems], F8, isOutput=False)
    acc_p = nc.declare_dram_parameter("partial", [P, ACC_COLS], F32, isOutput=True)
    gram_p = nc.declare_dram_parameter("gram", [BLK, BLK], F32, isOutput=True)

    with tile.TileContext(nc) as tc:
        with (
            tc.tile_pool(name="io", bufs=6) as io_pool,
            tc.tile_pool(name="work", bufs=1) as work_pool,
            tc.tile_pool(name="accp", bufs=1) as acc_pool,
            tc.psum_pool(name="gr", bufs=2) as psum_pool,
        ):
            acc = acc_pool.tile([P, ACC_COLS], F32)
            gram = psum_pool.tile([BLK, BLK], F32, tag="g1")
            gram_sb = acc_pool.tile([BLK, BLK], F32)

            # PE warm-up: back-to-back matmuls on a zeroed tile keep
            # the PE busy past the HAM's 3.4 us activity window,
            # latching the clock gate to 2.4 GHz before data arrives
            # (cold 128-row blocks take 107 ns vs 56 ns warm).
            warm = acc_pool.tile([P, BLK], F8)
            wpsum = psum_pool.tile([BLK, BLK], F32, tag="gw")
            nc.vector.memset(warm[:], 0)
            for wi in range(64):
                nc.tensor.matmul(
                    wpsum[:], warm[:], warm[:],
                    start=(wi == 0), stop=(wi == 63),
                )

            mm = 0
            ofs_pe = ofs_act = 0
            act_j = 0
            for kind, t, c0, nchunk in STREAM:
                if kind == "pe":
                    z = BLK * nchunk
                    ab = io_pool.tile([P, z], F8, tag="pe", bufs=12)
                    nc.sync.dma_start(
                        out=ab[:],
                        in_=pe_p[ofs_pe : ofs_pe + P * z].rearrange(
                            "(p m) -> p m", p=P
                        ),
                    )
                    ofs_pe += P * z
                    for b in range(nchunk):
                        blk = ab[:, b * BLK : (b + 1) * BLK]
                        nc.tensor.matmul(
                            gram[:], blk, blk,
                            start=(mm == 0), stop=(mm == N_MM - 1),
                        )
                        mm += 1
                else:
                    ab = io_pool.tile([P, 2 * FREE], F8, tag="act", bufs=4)
                    nc.sync.dma_start(
                        out=ab[:],
                        in_=act_p[ofs_act : ofs_act + P * 2 * FREE].rearrange(
                            "(p m) -> p m", p=P
                        ),
                    )
                    ofs_act += P * 2 * FREE
                    d = work_pool.tile([P, FREE], F16, tag="d", bufs=2)
                    nc.vector.tensor_sub(d[:], ab[:, 0:FREE], ab[:, FREE : 2 * FREE])
                    w = work_pool.tile([P, FREE], F16, tag="w", bufs=2)
                    nc.scalar.activation(
                        w[:], d[:],
                        mybir.ActivationFunctionType.Square,
                        accum_out=acc[:, act_j : act_j + 1],
                    )
                    if act_j < N_S2:
                        w2 = work_pool.tile([P, FREE], F16, tag="w2", bufs=2)
                        nc.scalar.activation(
                            w2[:], w[:],
                            mybir.ActivationFunctionType.Square,
                            accum_out=acc[:, N_ACT_TILES + act_j : N_ACT_TILES + act_j + 1],
                        )
                    act_j += 1

            # gram close: PSUM -> SBUF on the (idle) DVE, then write out.
            nc.vector.tensor_copy(gram_sb[:], gram[:])
            nc.sync.dma_start(out=gram_p[:], in_=gram_sb[:])
            nc.sync.dma_start(out=acc_p[:], in_=acc[:])
    nc.finalize()
    return nc


_NC = None


def _get_nc():
    global _NC
    if _NC is None:
        _NC = _build()
    return _NC


def _pack_gram_cols(o_t, t_t):
    """Whole row-tile -> chunked gram layout [P, N_CHUNK*BLK] fp8."""
    pad = GFREE - FREE
    o_p = np.pad(o_t, ((0, 0), (0, pad))).reshape(P, N_CHUNK, CH)
    t_p = np.pad(t_t, ((0, 0), (0, pad))).reshape(P, N_CHUNK, CH)
    return np.stack([o_p, t_p], axis=2).reshape(P, N_CHUNK * BLK)


def _shard_inputs(output, target):
    import ml_dtypes  # noqa: F401  (float8 numpy dtype support)

    output = np.asarray(output)
    target = np.asarray(target)
    f8np = mybir.dt.np(F8)
    in_maps = []
    for ci in range(N_CORES):
        sl = slice(ci * ROWS_PER_CORE, (ci + 1) * ROWS_PER_CORE)
        # center before fp8 quantization: (o-t) is shift-invariant and
        # fp8(x-0.5) has ~4x less quantization bias than fp8(x)
        o8 = (output[sl].astype(np.float32) - np.float32(0.5)).astype(f8np)
        t8 = (target[sl].astype(np.float32) - np.float32(0.5)).astype(f8np)
        o8 = o8.reshape(N_TILES, P, FREE)
        t8 = t8.reshape(N_TILES, P, FREE)
        pe_blocks = [
            _pack_gram_cols(o8[t], t8[t]).reshape(-1) for t in range(N_PE_TILES)
        ]
        act_blocks = [
            np.concatenate([o8[t], t8[t]], axis=1).reshape(-1)
            for t in range(N_PE_TILES, N_TILES)
        ]
        in_maps.append(
            {
                "comb_pe": np.concatenate(pe_blocks),
                "comb_act": np.concatenate(act_blocks),
            }
        )
    return in_maps


def run_device(output, target, trace=False):
    in_maps = _shard_inputs(output, target)
    res = run_bass_kernel_spmd(_get_nc(), in_maps, list(range(N_CORES)), trace=trace)
    partials = [
        (res.results[i]["partial"], res.results[i]["gram"]) for i in range(N_CORES)
    ]
    return partials, res


def _gram_s1(g64):
    dg = np.diag(g64)
    return dg[:CH].sum() + dg[CH:].sum() - 2.0 * np.diag(g64[:CH, CH:]).sum()


def _reduce(partials):
    s1 = s2 = 0.0
    for p, g in partials:
        p64 = p.astype(np.float64)
        s1 += _gram_s1(g.astype(np.float64))
        s1 += p64[:, :N_ACT_TILES].sum()
        s2 += p64[:, N_ACT_TILES:].sum()
    s2 *= S2_SCALE
    c = 4.0 * float(np.float32(1.0 / COLS))  # match reference's f32 sigma
    total = c * s1 - (c * c / 2.0) * s2
    n = float(ROWS) * float(COLS)
    return np.array(total / n, dtype=np.float32)


def kernel(output, target):
    partials, _ = run_device(output, target)
    return _reduce(partials)
